# revision 1
# baseline (speedup 1.0000x reference)
"""Sparse 2D-sliding-window + global-token attention block on 8 TRN2 NeuronCores.

Data-parallel over batch (B=8 -> one batch element per core, zero collectives).
Token order per core: 1024 patches (8 tiles of 128 = 4 grid rows), 8 specials last.

Phase A: QKV in bf16 (lhsT = X^T tiles, rhs = W^T) with j-pair double-bank PSUM,
  RMS-norm + RoPE in row layout (norm weights folded into cos/sin tables,
  rstd applied as the last elementwise pass on GpSimd so VectorE keeps the
  2x-mode passes), PE-transpose into qT/kT [d, tok], 4 transposes per PSUM
  bank with one batched drain copy.

Phase B: per head-PAIR p (heads 2p, 2p+1 live in partition halves of chunk p):
  - S^T per k-tile s against its tight 320-wide q-window, both heads as two
    row-group matmuls sharing one PSUM double-bank; special-query scores,
    special-key scores, and special-special scores ride spare columns of the
    same banks so ONE exp per (pair, s) converts everything.
  - mask multiply (single shared [128,320] interior mask tile) split across
    Vector and GpSimd.
  - PV computes O (not O^T): out[q-tile, 65] = P^T-slice^T @ [V | 1], so the
    softmax denominator lands per-PARTITION in column 64: one strided
    reciprocal + per-tile tensor_scalar normalize, then PE-transpose into
    oT [d, tok] for the out-projection.

Phase C: out projection consuming oT as lhsT, j-pair double banks, one copy +
  one DMA per m-tile.
"""

import numpy as np
import ml_dtypes

B, N, DIM, HEADS, HD = 8, 1032, 1024, 16, 64
SPECIAL, GRID, WINDOW = 8, 32, 3
NP = 1024          # patch tokens
P = 128
NT = NP // P       # 8 patch tiles (4 grid rows each)
NC_ = DIM // P     # 8 contraction chunks
NPAIR = HEADS // 2
EPS = 1e-6
bf16 = ml_dtypes.bfloat16

# st2 per-half-bank layout: [0:w) window scores, [w:w+8) special-query
# scores (both full-partition), s==7 adds [w+8:w+16) special x special on
# partitions 0:8. Special-key scores live in a per-pair quad bank.

_COMPILED = None


def _w01(s):
    # left edge snapped to the previous tile boundary so every PV matmul
    # output starts at partition 0 (outputs >32 partitions must start at 0)
    return max(0, (s - 1) * P), min(NP, (4 * s + 7) * GRID)


def _build():
    from contextlib import ExitStack
    import concourse.bass as bass
    import concourse.tile as tile
    from concourse import bacc, mybir
    from concourse.masks import make_identity

    dt = mybir.dt
    AF = mybir.ActivationFunctionType
    MUL = mybir.AluOpType.mult
    ADD = mybir.AluOpType.add

    nc = bacc.Bacc()

    xT = nc.declare_dram_parameter("xT", [P, NC_, N], dt.bfloat16, isOutput=False)
    wqkv = nc.declare_dram_parameter("wqkv", [P, NC_, 3 * DIM], dt.bfloat16, isOutput=False)
    wo = nc.declare_dram_parameter("wo", [P, NC_, DIM], dt.bfloat16, isOutput=False)
    cosq = nc.declare_dram_parameter("cosq", [P, NT + 1, HD], dt.bfloat16, isOutput=False)
    sinq = nc.declare_dram_parameter("sinq", [P, NT + 1, HD], dt.bfloat16, isOutput=False)
    cosk = nc.declare_dram_parameter("cosk", [P, NT + 1, HD], dt.bfloat16, isOutput=False)
    sink = nc.declare_dram_parameter("sink", [P, NT + 1, HD], dt.bfloat16, isOutput=False)
    msk = nc.declare_dram_parameter("msk", [P, 352], dt.bfloat16, isOutput=False)
    out = nc.declare_dram_parameter("out", [N, DIM], dt.float32, isOutput=True)

    def mslice(i):
        return slice(i * P, i * P + (P if i < NT else SPECIAL))

    def mp(i):
        return P if i < NT else SPECIAL

    with ExitStack() as ctx:
        ctx.enter_context(nc.allow_low_precision(reason="bf16 compute validated against f32 reference"))
        tc = ctx.enter_context(tile.TileContext(nc))
        persist = ctx.enter_context(tc.tile_pool(name="persist", bufs=1))
        temps = ctx.enter_context(tc.tile_pool(name="temps", bufs=3))

        # ---- resident SBUF tensors -------------------------------------
        wq_sb = persist.tile([P, NC_, 3 * DIM], dt.bfloat16)
        wo_sb = persist.tile([P, NC_, DIM], dt.bfloat16)
        tab = {}
        for nm, ap in (("cosq", cosq), ("sinq", sinq), ("cosk", cosk), ("sink", sink)):
            tab[nm] = persist.tile([P, NT + 1, HD], dt.bfloat16, tag=f"tab_{nm}", name=f"tab_{nm}")
        msk_sb = persist.tile([P, 352], dt.bfloat16)

        def load_weights_interleaved(xT_sb):
            for c in range(NC_):
                nc.scalar.dma_start(xT_sb[:, c, :], xT[:, c, :])
                nc.sync.dma_start(wq_sb[:, c, 0:1536], wqkv[:, c, 0:1536])
                nc.gpsimd.dma_start(wq_sb[:, c, 1536:3072], wqkv[:, c, 1536:3072])
                if c == 2:
                    for nm, ap in (("cosq", cosq), ("sinq", sinq),
                                   ("cosk", cosk), ("sink", sink)):
                        nc.sync.dma_start(tab[nm][:], ap[:])
                    nc.sync.dma_start(msk_sb[:], msk[:])

        qT_sb = persist.tile([P, NC_, N], dt.bfloat16, tag="qT")
        kT_sb = persist.tile([P, NC_, N], dt.bfloat16, tag="kT")
        oT_sb = persist.tile([P, NC_, N], dt.bfloat16, tag="oT")
        # V with interleaved ones column: [128, 9 s-tiles, 16 heads, 65]
        v_sb = persist.tile([P, NT + 1, HEADS, HD + 1], dt.bfloat16, tag="v")
        nc.vector.memset(v_sb[:, :, :, HD : HD + 1], 1.0)

        # specials' V replicated at partition bases 0/32/64/96 (quad rows)
        v_spk = persist.tile([P, HEADS, HD + 1], dt.bfloat16, tag="vspk")
        nc.vector.memset(v_spk[:, :, HD : HD + 1], 1.0)
        # kT specials zero-padded to 32 columns so quad matmuls write full rows
        kspad = persist.tile([P, 2, 32], dt.bfloat16, tag="kspad")
        nc.vector.memset(kspad[:], 0.0)

        ident = persist.tile([P, P], dt.bfloat16, tag="ident")
        make_identity(nc, ident[:])
        eps_sb = persist.tile([P, 1], dt.float32, tag="eps")
        nc.vector.memset(eps_sb[:], EPS)

        # ---- phase A: QKV projection + RMS norm + RoPE + transpose -----
        with tc.tile_pool(name="psumA", bufs=2, space="PSUM") as psA, \
             tc.tile_pool(name="sbA", bufs=2) as sbA:
            xT_sb = sbA.tile([P, NC_, N], dt.bfloat16, tag="xT", bufs=1)
            load_weights_interleaved(xT_sb)
            tr_pending = []

            def flush_transposes():
                for (ii, rope, dstT) in tr_pending:
                    mm = mp(ii)
                    mss = mslice(ii)
                    for cg in range(2):  # chunk groups of 4
                        trb = psA.tile([P, 4, P], dt.bfloat16, tag="tra", bufs=4,
                                       name=f"tr{ii}_{cg}")
                        for c4 in range(4):
                            c2 = cg * 4 + c4
                            nc.tensor.transpose(
                                trb[:P, c4, :mm], rope[:mm, c2 * P : (c2 + 1) * P],
                                ident[:mm, :mm],
                            )
                        nc.vector.tensor_copy(
                            dstT[:, cg * 4 : cg * 4 + 4, mss], trb[:, :, :mm]
                        )
                tr_pending.clear()

            for i in [NT] + list(range(NT)):
                m = mp(i)
                raws = {}
                for jp in range(3):  # q, k, v j-pairs
                    ps2 = psA.tile([P, 2, 512], dt.float32, tag="qkv", bufs=2,
                                   name=f"qkv{i}_{jp}")
                    for c in range(NC_):
                        lhsT = xT_sb[:, c, mslice(i)]
                        for jj in range(2):
                            nc.tensor.matmul(
                                ps2[:m, jj, :],
                                lhsT,
                                wq_sb[:, c, (jp * 2 + jj) * 512 : (jp * 2 + jj + 1) * 512],
                                start=(c == 0),
                                stop=(c == NC_ - 1),
                            )
                    if jp == 2:
                        # V: one strided copy into [head, 65] layout
                        nc.scalar.copy(
                            v_sb[:m, i, :, 0:HD],
                            ps2[:m, :, :].rearrange("p j (h d) -> p (j h) d", h=8),
                        )
                        if i == NT:
                            for bb in (0, 32, 64, 96):
                                nc.scalar.copy(
                                    v_spk[bb : bb + SPECIAL, :, 0:HD],
                                    ps2[:SPECIAL, :, :].rearrange(
                                        "p j (h d) -> p (j h) d", h=8),
                                )
                    else:
                        raw = sbA.tile([P, DIM], dt.bfloat16, tag="raw", bufs=2,
                                         name=f"raw{i}_{jp}")
                        nc.scalar.copy(raw[:m, :], ps2[:m, :, :].rearrange("p j f -> p (j f)"))
                        raws[jp] = raw
                flush_transposes()
                for jp, (cosn, sinn, dstT) in ((0, ("cosq", "sinq", qT_sb)),
                                               (1, ("cosk", "sink", kT_sb))):
                    raw = raws[jp]
                    # RMS statistics
                    sq = sbA.tile([P, DIM], dt.bfloat16, tag="sq", bufs=2)
                    nc.scalar.activation(sq[:m], raw[:m], AF.Square)
                    ssum = temps.tile([P, HEADS], dt.float32, tag="ssum")
                    nc.vector.reduce_sum(
                        ssum[:m], sq[:m].rearrange("p (h d) -> p h d", h=HEADS),
                        axis=mybir.AxisListType.X,
                    )
                    rstd = temps.tile([P, HEADS], dt.float32, tag="rstd")
                    nc.scalar.activation(rstd[:m], ssum[:m], AF.Sqrt, bias=eps_sb[:m],
                                         scale=1.0 / HD)
                    rst = temps.tile([P, HEADS], dt.bfloat16, tag="rst")
                    nc.vector.reciprocal(rst[:m], rstd[:m])
                    # RoPE (vector 2x passes) then rstd scale on gpsimd
                    cosw = tab[cosn][:m, i, None, :].to_broadcast((m, HEADS, HD))
                    sin0 = tab[sinn][:m, i, None, 0 : HD // 2].to_broadcast((m, HEADS, HD // 2))
                    sin1 = tab[sinn][:m, i, None, HD // 2 : HD].to_broadcast((m, HEADS, HD // 2))
                    rv = raw[:m].rearrange("p (h two half) -> p h two half", h=HEADS, two=2)
                    tc_t = sbA.tile([P, DIM], dt.bfloat16, tag="tcos", bufs=2)
                    nc.vector.tensor_tensor(
                        tc_t[:m].rearrange("p (h d) -> p h d", h=HEADS),
                        raw[:m].rearrange("p (h d) -> p h d", h=HEADS), cosw, op=MUL,
                    )
                    ts_t = sbA.tile([P, DIM], dt.bfloat16, tag="tsin", bufs=2)
                    tsv = ts_t[:m].rearrange("p (h two half) -> p h two half", h=HEADS, two=2)
                    nc.vector.tensor_tensor(tsv[:, :, 0, :], rv[:, :, 1, :], sin0, op=MUL)
                    nc.vector.tensor_tensor(tsv[:, :, 1, :], rv[:, :, 0, :], sin1, op=MUL)
                    nc.vector.tensor_tensor(tc_t[:m], tc_t[:m], ts_t[:m], op=ADD)
                    rope = sbA.tile([P, DIM], dt.bfloat16, tag="rope", bufs=4)
                    nc.gpsimd.tensor_tensor(
                        rope[:m].rearrange("p (h d) -> p h d", h=HEADS),
                        tc_t[:m].rearrange("p (h d) -> p h d", h=HEADS),
                        rst[:m, :, None].to_broadcast((m, HEADS, HD)), op=MUL,
                    )
                    tr_pending.append((i, rope, dstT))
            flush_transposes()
            for c in range(NC_):
                nc.sync.dma_start(wo_sb[:, c, :], wo[:, c, :])

        # ---- phase B: banded attention, per head pair ------------------
        with tc.tile_pool(name="psumB", bufs=1, space="PSUM") as psB, \
             tc.tile_pool(name="ptp", bufs=9) as ptp, \
             tc.tile_pool(name="obp", bufs=2) as obp:
            O_sbs = []
            for p in range(NPAIR):
                hA, hB = 2 * p, 2 * p + 1
                ObA_t = psB.tile([P, 512], dt.float32, tag="ob", bufs=2, name=f"obA{p}")
                ObB_t = psB.tile([P, 512], dt.float32, tag="ob", bufs=2, name=f"obB{p}")
                ObS_t = psB.tile([P, 512], dt.float32, tag="obs", bufs=1, name=f"obS{p}")
                ObA = ObA_t[:, 0 : 7 * (HD + 2)].rearrange("p (t c) -> p t c", c=HD + 2)
                ObB = ObB_t[:, 0 : 7 * (HD + 2)].rearrange("p (t c) -> p t c", c=HD + 2)
                ObS = ObS_t[:, 0 : 4 * (HD + 2)].rearrange("p (t c) -> p t c", c=HD + 2)
                spq = psB.tile([P, 4, P], dt.float32, tag="spq", bufs=1, name=f"spq{p}")
                Obs = {0: ObA, 1: ObB}
                pts = []
                pspk = [None]

                # stage kT specials into the zero-padded lhsT tile
                for half in (0, 1):
                    nc.vector.tensor_copy(
                        kspad[HD * half : HD * half + HD, half, 0:SPECIAL],
                        kT_sb[HD * half : HD * half + HD, p, NP : NP + SPECIAL],
                    )
                pk = ptp.tile([P, 4, P], dt.bfloat16, tag="pk", bufs=2, name=f"pk{p}")
                pspk[0] = pk

                def spq_group(j, p=p, spq=spq, pk=pk):
                    # special-key scores: quad bank, rows written via zero pad
                    for t in range(4 * j, 4 * j + 4):
                        for half in (0, 1):
                            base = 32 * (2 * j + half)
                            nc.tensor.matmul(
                                spq[base : base + 32, t % 4, :],
                                kspad[HD * half : HD * half + HD, half, :],
                                qT_sb[HD * half : HD * half + HD, p, t * P : (t + 1) * P],
                                start=True, stop=True,
                                tile_position=(HD * half, base),
                            )
                    nc.scalar.activation(pk[64 * j : 64 * j + 64, :, :],
                                         spq[64 * j : 64 * j + 64, :, :],
                                         AF.Exp, scale=0.125)

                def pv_tile(t, pts=pts, pspk=pspk, Obs=Obs, ObS=ObS, p=p):
                    for half, h in ((0, 2 * p), (1, 2 * p + 1)):
                        dst, slot = (Obs[half], t) if t < 7 else (ObS, 2 * half)
                        ss = [t] + [s for s in (t - 1, t + 1) if 0 <= s < NT]
                        for k, s in enumerate(ss):
                            w0, w1 = _w01(s)
                            qlo, qhi = max(t * P, w0), min((t + 1) * P, w1)
                            nc.tensor.matmul(
                                dst[qlo - t * P : qhi - t * P, slot, 0 : HD + 1],
                                pts[s][:, half, qlo - w0 : qhi - w0],
                                v_sb[:, s, h, :],
                                start=(k == 0), stop=False, skip_group_check=True,
                            )
                        base = 32 * (2 * (t // 4) + half)
                        nc.tensor.matmul(
                            dst[:P, slot, 0 : HD + 1],
                            pspk[0][base : base + SPECIAL, t % 4, :],
                            v_spk[base : base + SPECIAL, h, :],
                            start=False, stop=True, skip_group_check=True,
                            tile_position=(base, 0),
                        )

                for s in range(NT):
                    w0, w1 = _w01(s)
                    w = w1 - w0
                    st2 = psB.tile([P, 2, 512], dt.float32, tag="st", bufs=2, name=f"st{p}_{s}")
                    for half in (0, 1):
                        pb = HD * half
                        lhsT_k = kT_sb[pb : pb + HD, p, s * P : (s + 1) * P]
                        nc.tensor.matmul(
                            st2[:P, half, 0:w], lhsT_k, qT_sb[pb : pb + HD, p, w0:w1],
                            start=True, stop=True,
                        )
                        nc.tensor.matmul(
                            st2[:P, half, w : w + SPECIAL], lhsT_k,
                            qT_sb[pb : pb + HD, p, NP : NP + SPECIAL],
                            start=True, stop=True,
                        )
                        if s == NT - 1:
                            nc.tensor.matmul(
                                st2[0:SPECIAL, half, w + 8 : w + 16],
                                kT_sb[pb : pb + HD, p, NP : NP + SPECIAL],
                                qT_sb[pb : pb + HD, p, NP : NP + SPECIAL],
                                start=True, stop=True,
                            )
                    ptt = ptp.tile([P, 2, 368], dt.bfloat16, tag="pt", name=f"pt{p}_{s}")
                    nc.scalar.activation(ptt[:, :, 0 : w + 8], st2[:, :, 0 : w + 8],
                                         AF.Exp, scale=0.125)
                    if s == NT - 1:
                        nc.scalar.activation(
                            ptt[0:SPECIAL, :, w + 8 : w + 16],
                            st2[0:SPECIAL, :, w + 8 : w + 16], AF.Exp, scale=0.125,
                        )
                    eng = nc.gpsimd if s % 4 == 3 else nc.vector
                    moff = 128 if s == 0 else 0
                    eng.tensor_tensor(
                        ptt[:, :, 0:w], ptt[:, :, 0:w],
                        msk_sb[:, None, moff : moff + w].to_broadcast((P, 2, w)), op=MUL,
                    )
                    pts.append(ptt)
                    if s == 1:
                        spq_group(0)
                    if s == 5:
                        spq_group(1)
                    if s >= 2:
                        pv_tile(s - 2)
                pv_tile(NT - 2)
                pv_tile(NT - 1)
                # t8: special queries, accumulate over all k
                w7 = _w01(NT - 1)[1] - _w01(NT - 1)[0]
                for half, h in ((0, hA), (1, hB)):
                    for k, s in enumerate(range(NT)):
                        ws = _w01(s)[1] - _w01(s)[0]
                        nc.tensor.matmul(
                            ObS[0:SPECIAL, 2 * half + 1, 0 : HD + 1],
                            pts[s][:P, half, ws : ws + SPECIAL],
                            v_sb[:, s, h, :],
                            start=(k == 0), stop=False, skip_group_check=True,
                        )
                    nc.tensor.matmul(
                        ObS[0:SPECIAL, 2 * half + 1, 0 : HD + 1],
                        pts[NT - 1][0:SPECIAL, half, w7 + 8 : w7 + 16],
                        v_spk[0:SPECIAL, h, :],
                        start=False, stop=True, skip_group_check=True,
                    )

                # normalize (per-partition denominators); transposes deferred
                O_sb = obp.tile([P, NT + 1, P], dt.bfloat16, tag="osb", bufs=8,
                                name=f"osb{p}")
                O_sbs.append(O_sb)
                for half in (0, 1):
                    Ob = Obs[half]
                    recA = temps.tile([P, 7], dt.float32, tag="recA")
                    nc.vector.reciprocal(recA[:, :, None], Ob[:, :, HD : HD + 1])
                    recS7 = temps.tile([P, 1], dt.float32, tag="recS7")
                    nc.vector.reciprocal(recS7[:], ObS[:, 2 * half, HD : HD + 1])
                    recS8 = temps.tile([P, 1], dt.float32, tag="recS8")
                    nc.vector.reciprocal(
                        recS8[0:SPECIAL], ObS[0:SPECIAL, 2 * half + 1, HD : HD + 1]
                    )
                    nc.vector.tensor_tensor(
                        O_sb[:, 0:7, half * HD : half * HD + HD],
                        Ob[:, :, 0:HD],
                        recA[:, :, None].to_broadcast((P, 7, HD)), op=MUL,
                    )
                    nc.vector.tensor_tensor(
                        O_sb[:, 7, half * HD : half * HD + HD],
                        ObS[:, 2 * half, 0:HD],
                        recS7[:, 0:1].to_broadcast((P, HD)), op=MUL,
                    )
                    nc.vector.tensor_tensor(
                        O_sb[0:SPECIAL, NT, half * HD : half * HD + HD],
                        ObS[0:SPECIAL, 2 * half + 1, 0:HD],
                        recS8[0:SPECIAL, 0:1].to_broadcast((SPECIAL, HD)), op=MUL,
                    )
            # phase B2: transpose all pairs' O into oT
            for p in range(NPAIR):
                O_sb = O_sbs[p]
                for tg in range(2):
                    trb = psB.tile([P, 4, P], dt.bfloat16, tag="st", bufs=2, name=f"trO{p}_{tg}")
                    for t4 in range(4):
                        nc.tensor.transpose(
                            trb[:P, t4, :], O_sb[:, tg * 4 + t4, :], ident[:]
                        )
                    nc.vector.tensor_copy(
                        oT_sb[:, p, tg * 512 : (tg + 1) * 512],
                        trb[:].rearrange("p a b -> p (a b)"),
                    )
                trs = psB.tile([P, 4, P], dt.bfloat16, tag="st", bufs=2, name=f"trS{p}")
                nc.tensor.transpose(trs[:P, 0, :SPECIAL], O_sb[:SPECIAL, NT, :],
                                    ident[:SPECIAL, :SPECIAL])
                nc.vector.tensor_copy(oT_sb[:, p, NP:N], trs[:, 0, :SPECIAL])

        # ---- phase C: out projection -----------------------------------
        with tc.tile_pool(name="psumC", bufs=2, space="PSUM") as psC:
            for i in range(NT + 1):
                m = mp(i)
                row0 = SPECIAL + i * P if i < NT else 0
                py = psC.tile([P, 2, 512], dt.float32, tag="py", name=f"py{i}")
                for c in range(NC_):
                    for jj in range(2):
                        nc.tensor.matmul(
                            py[:m, jj, :],
                            oT_sb[:, c, mslice(i)],
                            wo_sb[:, c, jj * 512 : (jj + 1) * 512],
                            start=(c == 0),
                            stop=(c == NC_ - 1),
                        )
                y = temps.tile([P, DIM], dt.float32, tag="y", bufs=2)
                nc.scalar.copy(y[:m, :], py[:m, :, :].rearrange("p j f -> p (j f)"))
                nc.sync.dma_start(out[row0 : row0 + m, :], y[:m, :])

    nc.compile()
    return nc


def _get_compiled():
    global _COMPILED
    if _COMPILED is None:
        _COMPILED = _build()
    return _COMPILED


def _tile_cm(a2d, nchunks):
    """[K, F] -> [128, K//128, F] with element [p, c, f] = a2d[c*128+p, f]."""
    K, F = a2d.shape
    return np.ascontiguousarray(a2d.reshape(nchunks, P, F).transpose(1, 0, 2))


def _prep(freqs_cos, freqs_sin, qkv_w, out_w, norm_q_w, norm_k_w):
    perm = np.concatenate([np.arange(SPECIAL, N), np.arange(0, SPECIAL)])
    wqkv_t = _tile_cm(np.asarray(qkv_w, np.float32).T.astype(bf16), NC_)
    wo_t = _tile_cm(np.asarray(out_w, np.float32).T.astype(bf16), NC_)

    c_r = np.asarray(freqs_cos, np.float32)[perm]  # [1032, 64] in m-order
    s_r = np.asarray(freqs_sin, np.float32)[perm]
    h2 = HD // 2

    def fold(w):
        w = np.asarray(w, np.float32)
        cw = c_r * w[None, :]
        sw = np.empty_like(s_r)
        sw[:, :h2] = -s_r[:, :h2] * w[None, h2:]
        sw[:, h2:] = s_r[:, h2:] * w[None, :h2]
        return cw, sw

    cq, sq_ = fold(norm_q_w)
    ck, sk_ = fold(norm_k_w)

    def padtab(t):
        tp = np.zeros(((NT + 1) * P, HD), np.float32)
        tp[:N] = t
        return _tile_cm(tp.astype(bf16), NT + 1)

    # interior mask [128, 320]: k-row j of a tile vs q-window starting 3 grid
    # rows before the tile (edge tiles use offset slices of the same pattern)
    jj, ii = np.meshgrid(np.arange(P), np.arange(352), indexing="ij")
    m_int = (np.abs(jj // GRID - (ii // GRID - 4)) <= WINDOW) & (
        np.abs(jj % GRID - ii % GRID) <= WINDOW
    )
    return dict(
        wqkv=wqkv_t,
        wo=wo_t,
        cosq=padtab(cq),
        sinq=padtab(sq_),
        cosk=padtab(ck),
        sink=padtab(sk_),
        msk=m_int.astype(np.float32).astype(bf16),
    )


def make_in_maps(hidden_states, freqs_cos, freqs_sin, qkv_w, out_w, norm_q_w, norm_k_w):
    shared = _prep(freqs_cos, freqs_sin, qkv_w, out_w, norm_q_w, norm_k_w)
    perm = np.concatenate([np.arange(SPECIAL, N), np.arange(0, SPECIAL)])
    hs = np.asarray(hidden_states, np.float32)
    in_maps = []
    for b in range(B):
        xb = hs[b][perm]                       # [1032, 1024] m-order
        xT = _tile_cm(np.ascontiguousarray(xb.T).astype(bf16), NC_)  # [128, 8, 1032]
        in_maps.append(dict(shared, xT=xT))
    return in_maps


def kernel(hidden_states, freqs_cos, freqs_sin, qkv_w, out_w, norm_q_w, norm_k_w):
    from concourse.bass_utils import run_bass_kernel_spmd

    nc = _get_compiled()
    in_maps = make_in_maps(
        hidden_states, freqs_cos, freqs_sin, qkv_w, out_w, norm_q_w, norm_k_w
    )
    res = run_bass_kernel_spmd(nc, in_maps, core_ids=list(range(B)))
    return np.stack([np.asarray(res.results[i]["out"], np.float32) for i in range(B)])

